# revision 1
# baseline (speedup 1.0000x reference)
"""Trainium2 Bass kernel: contrastive loss with negative mining.

Math:
    centers  = mean over contiguous chunks of 8 rows               [n/8, d]
    x_pos    = x + 0.5*(center - x)        => |x - x_pos| = 0.5*|x - center|
    sim      = x @ x.T                                             [n, n]
    neg_idx  = argmax_j sim[i, j] excluding j in i's group-of-4
    d_ap     = mean_d |x - x_pos|,  d_an = mean_d |x - x_neg|
    loss     = sum( (1/8) * d_ap / (d_an + 1e-7) )

Distribution: data-parallel over rows, 8 NeuronCores, 1024 rows each.
Every core receives the full x.T (fp8) plus a bf16 copy of x in its own
DRAM, so no collectives are needed; per-row losses are returned and summed
on host.

Per core:
  - sim rows are fp8e4m3 DoubleRow matmuls (stationary = xT slice of this
    core's rows, moving = full xT) in 512-wide column strips, f32 PSUM
    accumulation over 8 k-pair blocks, evacuated to SBUF as bf16 (ScalarE).
  - Per strip, DVE max/max_index extract the top-8 values + indices per
    row.  A row's excluded group-of-4 spans at most 4 of its strip's
    top-8, so the best valid candidate always survives.
  - i-tiles are processed in two passes of 4 (the moving operand is read
    twice) so the first pass's negative-mining tail (candidate combine,
    x_neg gather, d_an) overlaps the second pass's matmuls.
  - Candidate combine is batched over 4 i-tiles: global indices,
    group-exclusion masking via compares against per-partition group
    bounds (input data), then argmax value + min-index-of-max reductions.
  - x_neg rows are gathered (bf16) from DRAM with a GPSIMD indirect DMA;
    d_an is a bf16 DVE subtract + ScalarE Abs+accumulate (f32 accum).
  - d_ap uses y = (I - blockdiag(ones(8,8)/8)) @ x_tile (bf16 matmuls,
    emitted last so they overlap the final tail) with ScalarE
    Abs+accumulate.
"""

import math

import ml_dtypes
import numpy as np

import concourse.bass as bass
import concourse.mybir as mybir
import concourse.tile as tile
from concourse import bacc
from concourse.bass import IndirectOffsetOnAxis
from concourse.bass_utils import run_bass_kernel_spmd

BF16 = mybir.dt.bfloat16
F32 = mybir.dt.float32
U32 = mybir.dt.uint32
ALU = mybir.AluOpType
ACTF = mybir.ActivationFunctionType
AXX = mybir.AxisListType.X

P = 128         # partitions / row-tile height
JS = 512        # similarity column-strip width
CHUNK = 8       # rows averaged per center
GROUP = 4       # negative-mining exclusion window
WEIGHT = 1.0 / 8
EPS = 1e-7
NEG_BIG = -1e30
BIGI = 65536.0  # index bias for the min-index-of-max trick


class Cfg:
    def __init__(self, n=8192, d=2048, cores=8, fp8=True):
        self.n, self.d, self.cores, self.fp8 = n, d, cores, fp8
        self.r = n // cores            # rows per core
        self.it = self.r // P          # i-tiles per core
        self.nj = n // JS              # column strips
        self.kb = d // P               # contraction blocks
        self.cw = min(d, JS)           # d-chunk width for the d_ap matmul
        self.ch = d // self.cw         # number of d-chunks
        self.nq = self.nj * 8          # candidates per i-tile
        self.gi = min(4, self.it)      # i-tiles per pass / combine batch
        assert n % (cores * P) == 0 and d % P == 0 and n % JS == 0
        assert d % self.cw == 0 and self.it % self.gi == 0


def _body(tc: tile.TileContext, cfg: Cfg, io: dict):
    nc = tc.nc
    ctxpools = {}

    def pool(name, bufs, space="SBUF"):
        if name not in ctxpools:
            ctxpools[name] = tc.alloc_tile_pool(name=name, bufs=bufs, space=space)
        return ctxpools[name]

    sim_dt = mybir.dt.float8e4 if cfg.fp8 else BF16

    # resident stationary xT slice: [128, KB*R], k-block major.
    # Chunked DMAs so the first matmuls can start before the full load lands.
    xs_sb = pool("xs", 1).tile([P, cfg.kb * cfg.r], sim_dt, name="xs_sb")
    for k in range(0, cfg.kb, 2):
        ke = min(k + 2, cfg.kb)
        nc.sync.dma_start(
            out=xs_sb[:, k * cfg.r:ke * cfg.r].rearrange(
                "p (a r) -> p a r", a=ke - k),
            in_=io["xs"][k * P:ke * P, :].rearrange("(a p) r -> p a r", p=P),
        )

    # resident bf16 x rows: phase-A moving operand AND d_an minuend
    xrb_sb = pool("xrb", 1).tile([P, cfg.it * cfg.d], BF16, name="xrb_sb")
    nc.sync.dma_start(
        out=xrb_sb[:].rearrange("p (a d) -> p a d", a=cfg.it),
        in_=io["xrb"][:, :].rearrange("(a p) d -> p a d", p=P),
    )

    psum = pool("ps", 8, space="PSUM")
    small = pool("small", 1)
    sap = small.tile([P, cfg.it * cfg.ch], F32, name="sap")    # sum|y| per chunk
    san = small.tile([P, cfg.it], F32, name="san")             # sum|x-xneg|
    idxall = small.tile([P, cfg.it], U32, name="idxall")       # neg indices
    # per-pass candidate tiles (separate so pass 0's combine does not
    # falsely depend on pass 1's writes)
    npass = cfg.it // cfg.gi
    cv_sb = [small.tile([P, cfg.gi * cfg.nq], BF16, name=f"cv{g}", tag=f"cv{g}")
             for g in range(npass)]
    ci_sb = [small.tile([P, cfg.gi * cfg.nq], U32, name=f"ci{g}", tag=f"ci{g}")
             for g in range(npass)]

    consts = pool("consts", 1)
    m2b_sb = consts.tile_from(io["m2b"])                     # [128,128] bf16
    offs_sb = consts.tile_from(io["offsw"])                  # [128,IT*NQ] f32
    g0_sb = consts.tile_from(io["g0w"])                      # [128,IT*NQ] f32
    g3_sb = consts.tile_from(io["g3w"])                      # [128,IT*NQ] f32

    xmp = pool("xm", 2)
    evac = pool("evac", 4)
    comb = pool("comb", 1)
    xneg_p = pool("xneg", 2)
    diff_p = pool("diff", 2)
    dabs_p = pool("dabs", 2)

    xs3 = xs_sb[:].rearrange("p (a r) -> p a r", a=cfg.kb)
    G = cfg.gi
    W = G * cfg.nq

    for a in range(0, cfg.it, G):
        # ---- sim strips for i-tiles [a, a+G) + per-strip top-8 ----
        for j in range(cfg.nj):
            xm_sb = xmp.tile([P, cfg.kb * JS], sim_dt, name="xm_sb")
            nc.sync.dma_start(
                out=xm_sb[:].rearrange("p (a b) -> p a b", a=cfg.kb),
                in_=io["xm"][:, j * JS:(j + 1) * JS].rearrange(
                    "(a p) b -> p a b", p=P),
            )
            xm3 = xm_sb[:].rearrange("p (a b) -> p a b", a=cfg.kb)
            for it in range(a, a + G):
                ps_s = psum.tile([P, JS], F32, name="ps_s", tag="ps")
                if cfg.fp8:
                    for k in range(0, cfg.kb, 2):
                        nc.tensor.matmul(
                            out=ps_s[:],
                            lhsT=xs3[:, k:k + 2, it * P:(it + 1) * P],
                            rhs=xm3[:, k:k + 2, :],
                            start=(k == 0), stop=(k == cfg.kb - 2),
                            perf_mode=mybir.MatmulPerfMode.DoubleRow,
                        )
                else:
                    for k in range(cfg.kb):
                        nc.tensor.matmul(
                            out=ps_s[:],
                            lhsT=xs_sb[:, k * cfg.r + it * P:
                                       k * cfg.r + (it + 1) * P],
                            rhs=xm_sb[:, k * JS:(k + 1) * JS],
                            start=(k == 0), stop=(k == cfg.kb - 1),
                        )
                sstrip = evac.tile([P, JS], BF16, name="sstrip")
                nc.scalar.copy(out=sstrip[:], in_=ps_s[:])
                q0 = ((it - a) * cfg.nj + j) * 8
                nc.vector.max(out=cv_sb[a // G][:, q0:q0 + 8], in_=sstrip[:])
                nc.vector.max_index(
                    out=ci_sb[a // G][:, q0:q0 + 8],
                    in_max=cv_sb[a // G][:, q0:q0 + 8],
                    in_values=sstrip[:],
                )

        # ---- batched candidate combine for this pass's i-tiles ----
        lo, hi = a * cfg.nq, (a + G) * cfg.nq
        cif = comb.tile([P, W], F32, name="cif")
        nc.vector.tensor_copy(out=cif[:], in_=ci_sb[a // G][:])
        gidx = comb.tile([P, W], F32, name="gidx")
        nc.vector.tensor_tensor(
            out=gidx[:], in0=cif[:], in1=offs_sb[:, lo:hi], op=ALU.add)
        b1 = comb.tile([P, W], F32, name="b1")
        nc.vector.tensor_tensor(
            out=b1[:], in0=gidx[:], in1=g0_sb[:, lo:hi], op=ALU.is_ge)
        b2 = comb.tile([P, W], F32, name="b2")
        nc.vector.tensor_tensor(
            out=b2[:], in0=gidx[:], in1=g3_sb[:, lo:hi], op=ALU.is_le)
        msk = comb.tile([P, W], F32, name="msk")
        nc.vector.scalar_tensor_tensor(
            out=msk[:], in0=b1[:], scalar=NEG_BIG, in1=b2[:],
            op0=ALU.mult, op1=ALU.mult)
        cvf = comb.tile([P, W], F32, name="cvf")
        nc.vector.tensor_copy(out=cvf[:], in_=cv_sb[a // G][:])
        mv = comb.tile([P, W], F32, name="mv")
        nc.vector.tensor_tensor(out=mv[:], in0=cvf[:], in1=msk[:], op=ALU.add)
        mv3 = mv[:].rearrange("p (g q) -> p g q", g=G)
        mxg = comb.tile([P, G], F32, name="mxg")
        nc.vector.tensor_reduce(out=mxg[:], in_=mv3, axis=AXX, op=ALU.max)
        sel = comb.tile([P, W], F32, name="sel")
        nc.vector.tensor_tensor(
            out=sel[:].rearrange("p (g q) -> p g q", g=G), in0=mv3,
            in1=mxg[:].to_broadcast([P, G, cfg.nq]), op=ALU.is_ge)
        pick = comb.tile([P, W], F32, name="pick")
        nc.vector.scalar_tensor_tensor(
            out=pick[:], in0=gidx[:], scalar=BIGI, in1=sel[:],
            op0=ALU.subtract, op1=ALU.mult)
        mng = comb.tile([P, G], F32, name="mng")
        nc.vector.tensor_reduce(
            out=mng[:], in_=pick[:].rearrange("p (g q) -> p g q", g=G),
            axis=AXX, op=ALU.min)
        idxg = comb.tile([P, G], F32, name="idxg")
        nc.vector.tensor_scalar(
            out=idxg[:], in0=mng[:], scalar1=BIGI, scalar2=None, op0=ALU.add)
        nc.vector.tensor_copy(out=idxall[:, a:a + G], in_=idxg[:])

        # ---- gather x_neg (bf16) + d_an for this pass's i-tiles ----
        for it in range(a, a + G):
            xneg = xneg_p.tile([P, cfg.d], BF16, name="xneg")
            nc.gpsimd.indirect_dma_start(
                out=xneg[:], out_offset=None,
                in_=io["xfb"][:, :],
                in_offset=IndirectOffsetOnAxis(ap=idxall[:, it:it + 1], axis=0),
                # an OOB index must not fault the device; skip it instead
                bounds_check=cfg.n - 1, oob_is_err=False,
            )
            diff = diff_p.tile([P, cfg.d], BF16, name="diff")
            nc.vector.tensor_tensor(
                out=diff[:], in0=xrb_sb[:, it * cfg.d:(it + 1) * cfg.d],
                in1=xneg[:], op=ALU.subtract,
            )
            dabs = dabs_p.tile([P, cfg.d], BF16, name="dabs")
            nc.scalar.activation(
                out=dabs[:], in_=diff[:], func=ACTF.Abs,
                accum_out=san[:, it:it + 1],
            )

    # ---- d_ap (emitted last; PE work overlaps the final pass's tail):
    #      y = M2 @ x_tile, sum_d |y|  (bf16 matmuls) ----
    yabs = pool("yabs", 2)
    for it in range(cfg.it):
        for c in range(cfg.ch):
            ps_y = psum.tile([P, cfg.cw], F32, name="ps_y", tag="ps")
            nc.tensor.matmul(
                out=ps_y[:], lhsT=m2b_sb[:],
                rhs=xrb_sb[:, it * cfg.d + c * cfg.cw:
                           it * cfg.d + (c + 1) * cfg.cw],
                start=True, stop=True,
            )
            y_sc = yabs.tile([P, cfg.cw], F32, name="y_sc")
            nc.scalar.activation(
                out=y_sc[:], in_=ps_y[:], func=ACTF.Abs,
                accum_out=sap[:, it * cfg.ch + c: it * cfg.ch + c + 1],
            )

    # ---- Final: per-row loss ----
    fin = pool("fin", 1)
    sap8 = fin.tile([P, cfg.it], F32, name="sap8")
    sap3 = sap[:].rearrange("p (a b) -> p a b", a=cfg.it)
    nc.vector.tensor_reduce(out=sap8[:], in_=sap3, axis=AXX, op=ALU.add)
    t1 = fin.tile([P, cfg.it], F32, name="t1")
    nc.vector.tensor_scalar(
        out=t1[:], in0=san[:], scalar1=1.0 / cfg.d, scalar2=EPS,
        op0=ALU.mult, op1=ALU.add,
    )
    rec = fin.tile([P, cfg.it], F32, name="rec")
    nc.vector.reciprocal(out=rec[:], in_=t1[:])
    t2 = fin.tile([P, cfg.it], F32, name="t2")
    nc.vector.tensor_tensor(out=t2[:], in0=sap8[:], in1=rec[:], op=ALU.mult)
    lossv = fin.tile([P, cfg.it], F32, name="lossv")
    nc.vector.tensor_scalar(
        out=lossv[:], in0=t2[:], scalar1=0.5 * WEIGHT / cfg.d, scalar2=None,
        op0=ALU.mult,
    )
    nc.sync.dma_start(out=io["loss_part"][:, :], in_=lossv[:])
    nc.sync.dma_start(out=io["nidx"][:, :], in_=idxall[:])

    for p in reversed(list(ctxpools.values())):
        p.release()


def build(cfg: Cfg) -> bass.Bass:
    nc = bacc.Bacc("TRN2", target_bir_lowering=False, debug=False)
    sim_dt = mybir.dt.float8e4 if cfg.fp8 else BF16
    nqw = cfg.it * cfg.nq
    io = {
        "xm": nc.dram_tensor("xm", [cfg.d, cfg.n], sim_dt, kind="ExternalInput").ap(),
        "xs": nc.dram_tensor("xs", [cfg.d, cfg.r], sim_dt, kind="ExternalInput").ap(),
        "xrb": nc.dram_tensor("xrb", [cfg.r, cfg.d], BF16, kind="ExternalInput").ap(),
        "xfb": nc.dram_tensor("xfb", [cfg.n, cfg.d], BF16, kind="ExternalInput").ap(),
        "m2b": nc.dram_tensor("m2b", [P, P], BF16, kind="ExternalInput").ap(),
        "offsw": nc.dram_tensor("offsw", [P, nqw], F32, kind="ExternalInput").ap(),
        "g0w": nc.dram_tensor("g0w", [P, nqw], F32, kind="ExternalInput").ap(),
        "g3w": nc.dram_tensor("g3w", [P, nqw], F32, kind="ExternalInput").ap(),
        "loss_part": nc.dram_tensor("loss_part", [P, cfg.it], F32, kind="ExternalOutput").ap(),
        "nidx": nc.dram_tensor("nidx", [P, cfg.it], U32, kind="ExternalOutput").ap(),
    }
    with tile.TileContext(nc) as tc:
        _body(tc, cfg, io)
    nc.compile()
    return nc


def make_in_maps(cfg: Cfg, x: np.ndarray) -> list[dict]:
    x = np.ascontiguousarray(x, dtype=np.float32)
    sim_np = ml_dtypes.float8_e4m3 if cfg.fp8 else ml_dtypes.bfloat16
    xt_q = np.ascontiguousarray(x.T.astype(sim_np))
    x_bf = x.astype(ml_dtypes.bfloat16)

    m2 = np.eye(P, dtype=np.float32)
    for c in range(P // CHUNK):
        m2[c * CHUNK:(c + 1) * CHUNK, c * CHUNK:(c + 1) * CHUNK] -= 1.0 / CHUNK
    m2b = m2.astype(ml_dtypes.bfloat16)

    # per-candidate global column offset (same pattern for every i-tile block)
    offs1 = np.zeros(cfg.nq, dtype=np.float32)
    for j in range(cfg.nj):
        offs1[j * 8:(j + 1) * 8] = j * JS
    offsw = np.broadcast_to(
        np.tile(offs1, cfg.it), (P, cfg.it * cfg.nq)).copy()

    pvec = np.arange(P, dtype=np.float32)
    in_maps = []
    for c in range(cfg.cores):
        g0w = np.zeros((P, cfg.it * cfg.nq), dtype=np.float32)
        for it in range(cfg.it):
            col = c * cfg.r + it * P + (pvec // GROUP) * GROUP
            g0w[:, it * cfg.nq:(it + 1) * cfg.nq] = col[:, None]
        in_maps.append({
            "xm": xt_q,
            "xs": np.ascontiguousarray(xt_q[:, c * cfg.r:(c + 1) * cfg.r]),
            "xrb": np.ascontiguousarray(x_bf[c * cfg.r:(c + 1) * cfg.r]),
            "xfb": x_bf,
            "m2b": m2b,
            "offsw": offsw,
            "g0w": g0w,
            "g3w": g0w + (GROUP - 1),
        })
    return in_maps


def reduce_outputs(cfg: Cfg, results: list[dict]) -> np.ndarray:
    total = 0.0
    for res in results:
        total += float(res["loss_part"].astype(np.float64).sum())
    return np.float32(total)


def run(cfg: Cfg, x: np.ndarray, trace: bool = False):
    nc = build(cfg)
    in_maps = make_in_maps(cfg, x)
    out = run_bass_kernel_spmd(nc, in_maps, list(range(cfg.cores)), trace=trace)
    return out


def kernel(x: np.ndarray) -> np.ndarray:
    cfg = Cfg(n=8192, d=2048, cores=8)
    last_err = None
    for _ in range(3):
        try:
            out = run(cfg, x)
            return reduce_outputs(cfg, out.results)
        except Exception as e:  # transient device errors: rebuild + retry
            last_err = e
    raise last_err



# revision 21
# speedup vs baseline: 1.1122x; 1.1122x over previous
"""Trainium2 Bass kernel: contrastive loss with negative mining.

Math:
    centers  = mean over contiguous chunks of 8 rows               [n/8, d]
    x_pos    = x + 0.5*(center - x)        => |x - x_pos| = 0.5*|x - center|
    sim      = x @ x.T                                             [n, n]
    neg_idx  = argmax_j sim[i, j] excluding j in i's group-of-4
    d_ap     = mean_d |x - x_pos|,  d_an = mean_d |x - x_neg|
    loss     = sum( (1/8) * d_ap / (d_an + 1e-7) )

Distribution: data-parallel over rows, 8 NeuronCores, 1024 rows each; no
collectives (each core gets full x.T in fp8), per-row losses summed on host.

Key layout trick: the similarity COLUMNS are rolled by -core*1024 on the
host, so every core sees its own 1024 rows as columns [0, 1024).  The
group-of-4 exclusion window for i-tile `it` is then the compile-time column
range [it*128, it*128+128) (strip it//4), identical on every core, which
lets the exclusion mask be fused into the PSUM evacuation.  x_neg is
gathered from an identically-rolled copy of x, so indices line up.

Per core:
  - sim rows: fp8e4m3 DoubleRow matmuls (stationary = own 128-row k-pair
    slice, moving = 512-wide strip of rolled x.T), f32 PSUM accumulation
    over 8 k-pair blocks.  Strip-major inner loop -> one PSUM bank per
    strip, 7-bank rotation (bank 8 is reserved for d_ap).
  - PSUM evac is a single DVE tensor_tensor_reduce per strip: adds the
    exclusion mask (own strips only), writes the f32 strip to SBUF AND
    emits the per-strip row max as accum_out.  No ScalarE copy, no MAX8.
  - Per (i-tile, strip-quarter): one max_index scan over the retained
    [128, 2048] f32 quarter finds the argmax position; quarters are
    combined with a min-index-of-max trick (ties resolve to the first
    column, matching jnp.argmax).
  - x_neg rows are gathered (bf16) from rolled-x DRAM with a GPSIMD
    indirect DMA; d_an = GPSIMD subtract + ScalarE Abs+accumulate.
  - d_ap uses y = (I - blockdiag(ones(8,8)/8)) @ x_tile (bf16 matmuls)
    interleaved one-per-block into the sim stream so the PE never idles.
  - Input DMAs are spread across the sync/scalar/vector/gpsimd queues so
    the first matmul issues within a few microseconds.
"""

import ml_dtypes
import numpy as np

import concourse.bass as bass
import concourse.mybir as mybir
import concourse.tile as tile
from concourse import bacc
from concourse.bass import IndirectOffsetOnAxis
from concourse.bass_utils import run_bass_kernel_spmd

BF16 = mybir.dt.bfloat16
F32 = mybir.dt.float32
U32 = mybir.dt.uint32
ALU = mybir.AluOpType
ACTF = mybir.ActivationFunctionType
AXX = mybir.AxisListType.X

P = 128         # partitions / row-tile height
JS = 512        # similarity column-strip width
CHUNK = 8       # rows averaged per center
GROUP = 4       # negative-mining exclusion window
WEIGHT = 1.0 / 8
EPS = 1e-7
NEG_BIG = -1e30
BIGI = 65536.0  # index bias for the min-index-of-max trick


class Cfg:
    def __init__(self, n=8192, d=2048, cores=8, fp8=True):
        self.n, self.d, self.cores = n, d, cores
        self.fp8 = True                # fp8 DoubleRow is the only path
        self.r = n // cores            # rows per core
        self.it = self.r // P          # i-tiles per core (8)
        self.ns = n // JS              # column strips (16)
        self.kb = d // P               # contraction blocks (16)
        self.nq = 4                    # strips per quarter-group
        self.qg = self.ns // self.nq   # quarter-groups (4)
        self.qw = self.nq * JS         # quarter width (2048)
        assert n % (cores * P) == 0 and d % (2 * P) == 0 and n % JS == 0


def _body(tc: tile.TileContext, cfg: Cfg, io: dict):
    nc = tc.nc
    ctxpools = {}

    def pool(name, bufs, space="SBUF"):
        if name not in ctxpools:
            ctxpools[name] = tc.alloc_tile_pool(name=name, bufs=bufs, space=space)
        return ctxpools[name]

    FP8 = mybir.dt.float8e4
    KB, IT, NS, NQ, QG, QW, D = (
        cfg.kb, cfg.it, cfg.ns, cfg.nq, cfg.qg, cfg.qw, cfg.d)

    # ---- resident inputs; DMAs split across the three HW queues
    #      (scalar / gpsimd / sync) so the first matmul's dependencies
    #      (xs k-pairs + strip 0) land within a few microseconds ----
    xs_sb = pool("xs", 1).tile([P, KB * cfg.r], FP8, name="xs_sb")

    def load_xs(k, eng):
        eng.dma_start(
            out=xs_sb[:, k * cfg.r:(k + 2) * cfg.r].rearrange(
                "p (a r) -> p a r", a=2),
            in_=io["xs"][k * P:(k + 2) * P, :].rearrange("(a p) r -> p a r", p=P),
        )

    xmp = pool("xm", 7)
    xm_t = {}

    def load_strip(s, eng):
        t = xmp.tile([P, KB * JS], FP8, name=f"xm{s}", tag="xm")
        eng.dma_start(
            out=t.rearrange("p (a b) -> p a b", a=KB),
            in_=io["xm"][:, s * JS:(s + 1) * JS].rearrange(
                "(a p) b -> p a b", p=P),
        )
        xm_t[s] = t.rearrange("p (a b) -> p a b", a=KB)

    consts = pool("consts", 1)
    maskw_sb = consts.tile([P, 896], F32, name="maskw_sb")
    zero8_sb = consts.tile([P, 8], BF16, name="zero8_sb")
    nc.vector.memset(zero8_sb[:], 0.0)
    m2b_sb = consts.tile([P, P], BF16, name="m2b_sb")
    qoff_sb = consts.tile([P, QG], F32, name="qoff_sb")
    xrb_sb = pool("xrb", 1).tile([P, IT * D], BF16, name="xrb_sb")

    # scalar queue: xs even k-pairs, then small consts
    for k in (0, 4, 8, 12):
        load_xs(k, nc.scalar)
    nc.scalar.dma_start(out=m2b_sb[:], in_=io["m2b"][:, :])
    nc.scalar.dma_start(out=qoff_sb[:], in_=io["qoffw"][:, :])
    # gpsimd queue: xs k-pairs 1,3 + mask, then xrb chunks
    load_xs(2, nc.gpsimd)
    nc.gpsimd.dma_start(out=maskw_sb[:], in_=io["maskw"][:, :])
    load_xs(6, nc.gpsimd)
    for itc in range(IT):
        nc.gpsimd.dma_start(
            out=xrb_sb[:, itc * D:(itc + 1) * D],
            in_=io["xrb"][itc * P:(itc + 1) * P, :],
        )
    # sync queue: strip 0, the last xs k-pairs, strips 1-3 (strips 4+
    # are prefetched inside the main loop)
    load_strip(0, nc.sync)
    load_xs(10, nc.sync)
    load_xs(14, nc.sync)
    load_strip(1, nc.sync)
    load_strip(2, nc.sync)
    load_strip(3, nc.sync)

    psum = pool("ps", 7, space="PSUM")
    psd = pool("psd", 1, space="PSUM")
    quart = pool("quart", 8)
    bc8p = pool("bc8", 3)
    ix8p = pool("ix8", 3)
    comb = pool("comb", 2)
    xneg_p = pool("xneg", 2)
    diff_p = pool("diff", 2)
    yabs = pool("yabs", 1)

    small = pool("small", 1)
    sm = small.tile([P, IT * NS], F32, name="sm")        # per-strip row max
    mq = small.tile([P, IT * QG], F32, name="mq")        # per-quarter max
    idxqf = small.tile([P, IT * QG], F32, name="idxqf")  # per-quarter argmax
    idxall = small.tile([P, IT], U32, name="idxall")     # final neg indices
    san = small.tile([P, IT], F32, name="san")           # sum|x-xneg|
    sap = small.tile([P, IT * QG], F32, name="sap")      # sum|y| per chunk
    lossv = small.tile([P, IT], F32, name="lossv")

    xs3 = xs_sb[:].rearrange("p (a r) -> p a r", a=KB)

    # d_ap jobs (it, chunk): one interleaved per sim block from block 6 on
    from collections import deque
    dapq = deque((i, c) for i in range(IT) for c in range(QG))

    def dap_emit(itd, cd):
        ps_y = psd.tile([P, JS], F32, name="ps_y", tag="psd")
        nc.tensor.matmul(
            out=ps_y[:], lhsT=m2b_sb[:],
            rhs=xrb_sb[:, itd * D + cd * JS: itd * D + (cd + 1) * JS],
            start=True, stop=True,
        )
        y_sc = yabs.tile([P, JS], F32, name="y_sc")
        nc.scalar.activation(
            out=y_sc[:], in_=ps_y[:], func=ACTF.Abs,
            accum_out=sap[:, itd * QG + cd: itd * QG + cd + 1],
        )

    def sim_strip(it, s, qt_out):
        """8 DoubleRow k-pair matmuls for (i-tile, strip) + fused evac:
        (psum + mask) -> f32 SBUF retained quarter, accum = row max."""
        ps_s = psum.tile([P, JS], F32, name="ps_s", tag="ps")
        for kp in range(KB // 2):
            nc.tensor.matmul(
                out=ps_s[:],
                lhsT=xs3[:, 2 * kp:2 * kp + 2, it * P:(it + 1) * P],
                rhs=xm_t[s][:, 2 * kp:2 * kp + 2, :],
                start=(kp == 0), stop=(kp == KB // 2 - 1),
                perf_mode=mybir.MatmulPerfMode.DoubleRow,
            )
        # own window [it*128, it*128+128) lives in strip it//4 (jq0 only):
        # those evacs add the exclusion mask on DVE; the rest are ScalarE
        # copies.  A DVE tensor_reduce then emits the per-strip row max.
        if s == it // 4:
            off = (it % 4) * P
            nc.vector.tensor_tensor(
                out=qt_out, in0=ps_s[:],
                in1=maskw_sb[:, 384 - off:384 - off + JS], op=ALU.add)
        else:
            nc.scalar.copy(out=qt_out, in_=ps_s[:])
        nc.vector.tensor_reduce(
            out=sm[:, it * NS + s: it * NS + s + 1],
            in_=qt_out.rearrange("p (a b) -> p a b", a=1),
            axis=AXX, op=ALU.max,
        )

    def scan_emit(it, jq, qt):
        """quarter argmax: one max_index scan over the [P, 2048] quarter"""
        q0 = it * QG + jq
        nc.vector.tensor_reduce(
            out=mq[:, q0:q0 + 1],
            in_=sm[:, it * NS + jq * NQ: it * NS + (jq + 1) * NQ
                   ].rearrange("p (a b) -> p a b", a=1),
            axis=AXX, op=ALU.max,
        )
        bc8 = bc8p.tile([P, 8], BF16, name="bc8", tag="bc8")
        nc.vector.tensor_scalar(
            out=bc8[:], in0=zero8_sb[:], scalar1=mq[:, q0:q0 + 1],
            scalar2=None, op0=ALU.add)
        ix8 = ix8p.tile([P, 8], U32, name="ix8", tag="ix8")
        nc.vector.max_index(out=ix8[:], in_max=bc8[:], in_values=qt[:])
        nc.vector.tensor_copy(out=idxqf[:, q0:q0 + 1], in_=ix8[:, 0:1])

    blk = 0

    def maybe_dap():
        nonlocal blk
        if blk >= 6 and dapq:
            dap_emit(*dapq.popleft())
        blk += 1

    # ---- jq0: strip-major (all i-tiles per strip) so full-rate matmuls
    #      need only strip 0 + xs resident.  Scans deferred into jq1. ----
    for s in range(NQ, 2 * NQ):     # prefetch jq1's strips
        load_strip(s, nc.sync)
    qt0 = {}
    for js in range(NQ):
        for it in range(IT):
            if js == 0:
                qt0[it] = quart.tile([P, QW], BF16, name=f"qt0_{it}", tag="q")
            sim_strip(it, js, qt0[it][:, js * JS:(js + 1) * JS])
            maybe_dap()

    # ---- jq1..3: i-tile-major blocks ----
    for jq in range(1, QG):
        for s in range((jq + 1) * NQ, min((jq + 2) * NQ, NS)):
            load_strip(s, nc.sync)
        for it in range(IT):
            if jq == 1:
                # deferred jq0 scan; MUST precede this block's evacs
                # (they recycle qt0[it]'s buffer)
                scan_emit(it, 0, qt0[it])
            qt = quart.tile([P, QW], BF16, name="qt", tag="q")
            for js in range(NQ):
                sim_strip(it, jq * NQ + js, qt[:, js * JS:(js + 1) * JS])
            maybe_dap()
            scan_emit(it, jq, qt)

            if jq == QG - 1:
                # ---- combine quarters, gather x_neg, d_an, final loss ----
                gidx = comb.tile([P, QG], F32, name="gidx", tag="c0")
                nc.vector.tensor_tensor(
                    out=gidx[:], in0=idxqf[:, it * QG:(it + 1) * QG],
                    in1=qoff_sb[:], op=ALU.add)
                mt = comb.tile([P, 1], F32, name="mt", tag="c1")
                nc.vector.tensor_reduce(
                    out=mt[:],
                    in_=mq[:, it * QG:(it + 1) * QG].rearrange(
                        "p (a b) -> p a b", a=1),
                    axis=AXX, op=ALU.max)
                ge = comb.tile([P, QG], F32, name="ge", tag="c2")
                nc.vector.tensor_tensor(
                    out=ge[:], in0=mq[:, it * QG:(it + 1) * QG],
                    in1=mt[:].to_broadcast([P, QG]), op=ALU.is_ge)
                pick = comb.tile([P, QG], F32, name="pick", tag="c3")
                nc.vector.scalar_tensor_tensor(
                    out=pick[:], in0=ge[:], scalar=-BIGI, in1=gidx[:],
                    op0=ALU.mult, op1=ALU.add)
                mn = comb.tile([P, 1], F32, name="mn", tag="c4")
                nc.vector.tensor_reduce(
                    out=mn[:], in_=pick[:].rearrange("p (a b) -> p a b", a=1),
                    axis=AXX, op=ALU.min)
                idxf = comb.tile([P, 1], F32, name="idxf", tag="c5")
                nc.vector.tensor_scalar(
                    out=idxf[:], in0=mn[:], scalar1=BIGI, scalar2=None,
                    op0=ALU.add)
                nc.vector.tensor_copy(out=idxall[:, it:it + 1], in_=idxf[:])

                xneg = xneg_p.tile([P, D], BF16, name="xneg")
                nc.gpsimd.indirect_dma_start(
                    out=xneg[:], out_offset=None,
                    in_=io["xfb"][:, :],
                    in_offset=IndirectOffsetOnAxis(
                        ap=idxall[:, it:it + 1], axis=0),
                    bounds_check=cfg.n - 1, oob_is_err=False,
                )
                diff = diff_p.tile([P, D], BF16, name="diff")
                nc.gpsimd.tensor_tensor(
                    out=diff[:], in0=xrb_sb[:, it * D:(it + 1) * D],
                    in1=xneg[:], op=ALU.subtract,
                )
                dabs = diff_p.tile([P, D], BF16, name="dabs", tag="dabs")
                nc.scalar.activation(
                    out=dabs[:], in_=diff[:], func=ACTF.Abs,
                    accum_out=san[:, it:it + 1],
                )

                sap8 = comb.tile([P, 1], F32, name="sap8", tag="c6")
                nc.vector.tensor_reduce(
                    out=sap8[:],
                    in_=sap[:, it * QG:(it + 1) * QG].rearrange(
                        "p (a b) -> p a b", a=1),
                    axis=AXX, op=ALU.add)
                t1 = comb.tile([P, 1], F32, name="t1", tag="c7")
                nc.vector.tensor_scalar(
                    out=t1[:], in0=san[:, it:it + 1], scalar1=1.0 / D,
                    scalar2=EPS, op0=ALU.mult, op1=ALU.add)
                rec = comb.tile([P, 1], F32, name="rec", tag="c8")
                nc.vector.reciprocal(out=rec[:], in_=t1[:])
                t2 = comb.tile([P, 1], F32, name="t2", tag="c9")
                nc.vector.tensor_tensor(
                    out=t2[:], in0=sap8[:], in1=rec[:], op=ALU.mult)
                nc.vector.tensor_scalar(
                    out=lossv[:, it:it + 1], in0=t2[:],
                    scalar1=0.5 * WEIGHT / D, scalar2=None, op0=ALU.mult)

    nc.sync.dma_start(out=io["loss_part"][:, :], in_=lossv[:])
    nc.sync.dma_start(out=io["nidx"][:, :], in_=idxall[:])

    for p in reversed(list(ctxpools.values())):
        p.release()


def build(cfg: Cfg) -> bass.Bass:
    nc = bacc.Bacc("TRN2", target_bir_lowering=False, debug=False)
    io = {
        "xm": nc.dram_tensor("xm", [cfg.d, cfg.n], mybir.dt.float8e4,
                             kind="ExternalInput").ap(),
        "xs": nc.dram_tensor("xs", [cfg.d, cfg.r], mybir.dt.float8e4,
                             kind="ExternalInput").ap(),
        "xrb": nc.dram_tensor("xrb", [cfg.r, cfg.d], BF16,
                              kind="ExternalInput").ap(),
        "xfb": nc.dram_tensor("xfb", [cfg.n, cfg.d], BF16,
                              kind="ExternalInput").ap(),
        "m2b": nc.dram_tensor("m2b", [P, P], BF16, kind="ExternalInput").ap(),
        "maskw": nc.dram_tensor("maskw", [P, 896], F32,
                                kind="ExternalInput").ap(),
        "qoffw": nc.dram_tensor("qoffw", [P, cfg.qg], F32,
                                kind="ExternalInput").ap(),
        "loss_part": nc.dram_tensor("loss_part", [P, cfg.it], F32,
                                    kind="ExternalOutput").ap(),
        "nidx": nc.dram_tensor("nidx", [P, cfg.it], U32,
                               kind="ExternalOutput").ap(),
    }
    with tile.TileContext(nc) as tc:
        _body(tc, cfg, io)
    nc.compile()
    return nc


def make_in_maps(cfg: Cfg, x: np.ndarray) -> list[dict]:
    x = np.ascontiguousarray(x, dtype=np.float32)
    xt_q = np.ascontiguousarray(x.T.astype(ml_dtypes.float8_e4m3))
    x_bf = x.astype(ml_dtypes.bfloat16)

    m2 = np.eye(P, dtype=np.float32)
    for c in range(P // CHUNK):
        m2[c * CHUNK:(c + 1) * CHUNK, c * CHUNK:(c + 1) * CHUNK] -= 1.0 / CHUNK
    m2b = m2.astype(ml_dtypes.bfloat16)

    # [zeros(384) | group-exclusion mask(128) | zeros(384)]; slicing a
    # 512-wide window places the mask at the i-tile's own column offset.
    maskw = np.zeros((P, 896), dtype=np.float32)
    pvec = np.arange(P)
    for jcol in range(GROUP):
        maskw[pvec, 384 + (pvec // GROUP) * GROUP + jcol] = NEG_BIG

    qoffw = np.broadcast_to(
        (np.arange(cfg.qg, dtype=np.float32) * cfg.qw), (P, cfg.qg)).copy()

    in_maps = []
    for c in range(cfg.cores):
        xm_c = np.ascontiguousarray(np.roll(xt_q, -c * cfg.r, axis=1))
        in_maps.append({
            "xm": xm_c,
            "xs": np.ascontiguousarray(xm_c[:, :cfg.r]),
            "xrb": np.ascontiguousarray(x_bf[c * cfg.r:(c + 1) * cfg.r]),
            "xfb": np.ascontiguousarray(np.roll(x_bf, -c * cfg.r, axis=0)),
            "m2b": m2b,
            "maskw": maskw,
            "qoffw": qoffw,
        })
    return in_maps


def reduce_outputs(cfg: Cfg, results: list[dict]) -> np.ndarray:
    total = 0.0
    for res in results:
        total += float(res["loss_part"].astype(np.float64).sum())
    return np.float32(total)


def run(cfg: Cfg, x: np.ndarray, trace: bool = False):
    nc = build(cfg)
    in_maps = make_in_maps(cfg, x)
    out = run_bass_kernel_spmd(nc, in_maps, list(range(cfg.cores)), trace=trace)
    return out


def kernel(x: np.ndarray) -> np.ndarray:
    cfg = Cfg(n=8192, d=2048, cores=8)
    last_err = None
    for _ in range(3):
        try:
            out = run(cfg, x)
            return reduce_outputs(cfg, out.results)
        except Exception as e:  # transient device errors: rebuild + retry
            last_err = e
    raise last_err


# revision 25
# speedup vs baseline: 1.1369x; 1.0222x over previous
"""Trainium2 Bass kernel: contrastive loss with negative mining.

Math:
    centers  = mean over contiguous chunks of 8 rows               [n/8, d]
    x_pos    = x + 0.5*(center - x)        => |x - x_pos| = 0.5*|x - center|
    sim      = x @ x.T                                             [n, n]
    neg_idx  = argmax_j sim[i, j] excluding j in i's group-of-4
    d_ap     = mean_d |x - x_pos|,  d_an = mean_d |x - x_neg|
    loss     = sum( (1/8) * d_ap / (d_an + 1e-7) )

Distribution: data-parallel over rows, 8 NeuronCores, 1024 rows each; no
collectives (each core gets full x.T in fp8), per-row losses summed on host.

Key layout trick: the similarity COLUMNS are rolled by -core*1024 on the
host, so every core sees its own 1024 rows as columns [0, 1024).  The
group-of-4 exclusion window for i-tile `it` is then the compile-time column
range [it*128, it*128+128) (strip it//4), identical on every core, which
lets the exclusion mask be fused into the PSUM evacuation.  x_neg is
gathered from an identically-rolled copy of x, so indices line up.

Per core:
  - sim rows: fp8e4m3 DoubleRow matmuls (stationary = own 128-row k-pair
    slice, moving = 512-wide strip of rolled x.T), f32 PSUM accumulation
    over 8 k-pair blocks.  Strip-major inner loop -> one PSUM bank per
    strip, 7-bank rotation (bank 8 is reserved for d_ap).
  - PSUM evac is a single DVE tensor_tensor_reduce per strip: adds the
    exclusion mask (own strips only), writes the f32 strip to SBUF AND
    emits the per-strip row max as accum_out.  No ScalarE copy, no MAX8.
  - Per (i-tile, strip-quarter): one max_index scan over the retained
    [128, 2048] f32 quarter finds the argmax position; quarters are
    combined with a min-index-of-max trick (ties resolve to the first
    column, matching jnp.argmax).
  - x_neg rows are gathered (bf16) from rolled-x DRAM with a GPSIMD
    indirect DMA; d_an = GPSIMD subtract + ScalarE Abs+accumulate.
  - d_ap uses y = (I - blockdiag(ones(8,8)/8)) @ x_tile (bf16 matmuls)
    interleaved one-per-block into the sim stream so the PE never idles.
  - Input DMAs are spread across the sync/scalar/vector/gpsimd queues so
    the first matmul issues within a few microseconds.
"""

import ml_dtypes
import numpy as np

import concourse.bass as bass
import concourse.mybir as mybir
import concourse.tile as tile
from concourse import bacc
from concourse.bass import IndirectOffsetOnAxis
from concourse.bass_utils import run_bass_kernel_spmd

BF16 = mybir.dt.bfloat16
F32 = mybir.dt.float32
U32 = mybir.dt.uint32
ALU = mybir.AluOpType
ACTF = mybir.ActivationFunctionType
AXX = mybir.AxisListType.X

P = 128         # partitions / row-tile height
JS = 512        # similarity column-strip width
CHUNK = 8       # rows averaged per center
GROUP = 4       # negative-mining exclusion window
WEIGHT = 1.0 / 8
EPS = 1e-7
NEG_BIG = -1e30
BIGI = 65536.0  # index bias for the min-index-of-max trick


class Cfg:
    def __init__(self, n=8192, d=2048, cores=8, fp8=True):
        self.n, self.d, self.cores = n, d, cores
        self.fp8 = True                # fp8 DoubleRow is the only path
        self.r = n // cores            # rows per core
        self.it = self.r // P          # i-tiles per core (8)
        self.ns = n // JS              # column strips (16)
        self.kb = d // P               # contraction blocks (16)
        self.nq = 4                    # strips per quarter-group
        self.qg = self.ns // self.nq   # quarter-groups (4)
        self.qw = self.nq * JS         # quarter width (2048)
        assert n % (cores * P) == 0 and d % (2 * P) == 0 and n % JS == 0


def _body(tc: tile.TileContext, cfg: Cfg, io: dict):
    nc = tc.nc
    ctxpools = {}

    def pool(name, bufs, space="SBUF"):
        if name not in ctxpools:
            ctxpools[name] = tc.alloc_tile_pool(name=name, bufs=bufs, space=space)
        return ctxpools[name]

    FP8 = mybir.dt.float8e4
    KB, IT, NS, NQ, QG, QW, D = (
        cfg.kb, cfg.it, cfg.ns, cfg.nq, cfg.qg, cfg.qw, cfg.d)

    # ---- resident inputs; DMAs split across the three HW queues
    #      (scalar / gpsimd / sync) so the first matmul's dependencies
    #      (xs k-pairs + strip 0) land within a few microseconds ----
    # xm/xs are host-swizzled so each partition's slice is one contiguous
    # run (8-16 KB) -> DMAs hit full HBM rate with few descriptors.
    xs_sb = pool("xs", 1).tile([P, KB * cfg.r], FP8, name="xs_sb")

    def load_xs(k, eng):
        eng.dma_start(
            out=xs_sb[:, k * cfg.r:(k + 2) * cfg.r],
            in_=io["xs"][:, k * cfg.r:(k + 2) * cfg.r],
        )

    xmp = pool("xm", 7)
    xm_t = {}

    def load_strip(s, eng):
        t = xmp.tile([P, KB * JS], FP8, name=f"xm{s}", tag="xm")
        eng.dma_start(out=t[:], in_=io["xm"][s * P:(s + 1) * P, :])
        xm_t[s] = t.rearrange("p (a b) -> p a b", a=KB)

    consts = pool("consts", 1)
    maskw_sb = consts.tile([P, 896], F32, name="maskw_sb")
    zero8_sb = consts.tile([P, 8], BF16, name="zero8_sb")
    nc.vector.memset(zero8_sb[:], 0.0)
    m2b_sb = consts.tile([P, P], BF16, name="m2b_sb")
    qoff_sb = consts.tile([P, QG], F32, name="qoff_sb")
    xrb_sb = pool("xrb", 1).tile([P, IT * D], BF16, name="xrb_sb")

    # scalar queue: xs even k-pairs, then small consts
    for k in (0, 4, 8, 12):
        load_xs(k, nc.scalar)
    nc.scalar.dma_start(out=m2b_sb[:], in_=io["m2b"][:, :])
    nc.scalar.dma_start(out=qoff_sb[:], in_=io["qoffw"][:, :])
    # gpsimd queue: xs k-pairs 1,3 + mask, then xrb chunks
    load_xs(2, nc.gpsimd)
    nc.gpsimd.dma_start(out=maskw_sb[:], in_=io["maskw"][:, :])
    load_xs(6, nc.gpsimd)
    for itc in range(IT):
        nc.gpsimd.dma_start(
            out=xrb_sb[:, itc * D:(itc + 1) * D],
            in_=io["xrb"][itc * P:(itc + 1) * P, :],
        )
    # sync queue: strip 0, the last xs k-pairs, strips 1-3 (strips 4+
    # are prefetched inside the main loop)
    load_strip(0, nc.sync)
    load_xs(10, nc.sync)
    load_xs(14, nc.sync)
    load_strip(1, nc.sync)
    load_strip(2, nc.sync)
    load_strip(3, nc.sync)

    psum = pool("ps", 7, space="PSUM")
    psd = pool("psd", 1, space="PSUM")
    quart = pool("quart", 8)
    bc8p = pool("bc8", 3)
    ix8p = pool("ix8", 3)
    comb = pool("comb", 2)
    xneg_p = pool("xneg", 2)
    diff_p = pool("diff", 2)
    yabs = pool("yabs", 1)

    small = pool("small", 1)
    sm = small.tile([P, IT * NS], F32, name="sm")        # per-strip row max
    mq = small.tile([P, IT * QG], F32, name="mq")        # per-quarter max
    idxqf = small.tile([P, IT * QG], F32, name="idxqf")  # per-quarter argmax
    idxall = small.tile([P, IT], U32, name="idxall")     # final neg indices
    san = small.tile([P, IT], F32, name="san")           # sum|x-xneg|
    sap = small.tile([P, IT * QG], F32, name="sap")      # sum|y| per chunk
    lossv = small.tile([P, IT], F32, name="lossv")

    xs3 = xs_sb[:].rearrange("p (a r) -> p a r", a=KB)

    # d_ap jobs (it, chunk): one interleaved per sim block from block 6 on
    from collections import deque
    dapq = deque((i, c) for i in range(IT) for c in range(QG))

    def dap_emit(itd, cd):
        ps_y = psd.tile([P, JS], F32, name="ps_y", tag="psd")
        nc.tensor.matmul(
            out=ps_y[:], lhsT=m2b_sb[:],
            rhs=xrb_sb[:, itd * D + cd * JS: itd * D + (cd + 1) * JS],
            start=True, stop=True,
        )
        y_sc = yabs.tile([P, JS], F32, name="y_sc")
        nc.scalar.activation(
            out=y_sc[:], in_=ps_y[:], func=ACTF.Abs,
            accum_out=sap[:, itd * QG + cd: itd * QG + cd + 1],
        )

    def sim_strip(it, s, qt_out):
        """8 DoubleRow k-pair matmuls for (i-tile, strip) + fused evac:
        (psum + mask) -> f32 SBUF retained quarter, accum = row max."""
        ps_s = psum.tile([P, JS], F32, name="ps_s", tag="ps")
        for kp in range(KB // 2):
            nc.tensor.matmul(
                out=ps_s[:],
                lhsT=xs3[:, 2 * kp:2 * kp + 2, it * P:(it + 1) * P],
                rhs=xm_t[s][:, 2 * kp:2 * kp + 2, :],
                start=(kp == 0), stop=(kp == KB // 2 - 1),
                perf_mode=mybir.MatmulPerfMode.DoubleRow,
            )
        # own window [it*128, it*128+128) lives in strip it//4 (jq0 only):
        # those evacs add the exclusion mask on DVE; the rest are ScalarE
        # copies.  A DVE tensor_reduce then emits the per-strip row max.
        if s == it // 4:
            off = (it % 4) * P
            nc.vector.tensor_tensor(
                out=qt_out, in0=ps_s[:],
                in1=maskw_sb[:, 384 - off:384 - off + JS], op=ALU.add)
        else:
            nc.scalar.copy(out=qt_out, in_=ps_s[:])
        nc.vector.tensor_reduce(
            out=sm[:, it * NS + s: it * NS + s + 1],
            in_=qt_out.rearrange("p (a b) -> p a b", a=1),
            axis=AXX, op=ALU.max,
        )

    def scan_emit(it, jq, qt):
        """quarter argmax: one max_index scan over the [P, 2048] quarter"""
        q0 = it * QG + jq
        nc.vector.tensor_reduce(
            out=mq[:, q0:q0 + 1],
            in_=sm[:, it * NS + jq * NQ: it * NS + (jq + 1) * NQ
                   ].rearrange("p (a b) -> p a b", a=1),
            axis=AXX, op=ALU.max,
        )
        bc8 = bc8p.tile([P, 8], BF16, name="bc8", tag="bc8")
        nc.vector.tensor_scalar(
            out=bc8[:], in0=zero8_sb[:], scalar1=mq[:, q0:q0 + 1],
            scalar2=None, op0=ALU.add)
        ix8 = ix8p.tile([P, 8], U32, name="ix8", tag="ix8")
        nc.vector.max_index(out=ix8[:], in_max=bc8[:], in_values=qt[:])
        nc.vector.tensor_copy(out=idxqf[:, q0:q0 + 1], in_=ix8[:, 0:1])

    blk = 0

    def maybe_dap():
        nonlocal blk
        if blk >= 6 and dapq:
            dap_emit(*dapq.popleft())
        blk += 1

    # ---- jq0: strip-major (all i-tiles per strip) so full-rate matmuls
    #      need only strip 0 + xs resident.  Scans deferred into jq1. ----
    for s in range(NQ, 2 * NQ):     # prefetch jq1's strips
        load_strip(s, nc.sync)
    qt0 = {}
    for js in range(NQ):
        for it in range(IT):
            if js == 0:
                qt0[it] = quart.tile([P, QW], BF16, name=f"qt0_{it}", tag="q")
            sim_strip(it, js, qt0[it][:, js * JS:(js + 1) * JS])
            maybe_dap()

    # ---- jq1..3: i-tile-major blocks ----
    for jq in range(1, QG):
        for s in range((jq + 1) * NQ, min((jq + 2) * NQ, NS)):
            load_strip(s, nc.sync)
        for it in range(IT):
            if jq == 1:
                # deferred jq0 scan; MUST precede this block's evacs
                # (they recycle qt0[it]'s buffer)
                scan_emit(it, 0, qt0[it])
            qt = quart.tile([P, QW], BF16, name="qt", tag="q")
            for js in range(NQ):
                sim_strip(it, jq * NQ + js, qt[:, js * JS:(js + 1) * JS])
            maybe_dap()
            scan_emit(it, jq, qt)

            if jq == QG - 1:
                # ---- combine quarters, gather x_neg, d_an, final loss ----
                gidx = comb.tile([P, QG], F32, name="gidx", tag="c0")
                nc.vector.tensor_tensor(
                    out=gidx[:], in0=idxqf[:, it * QG:(it + 1) * QG],
                    in1=qoff_sb[:], op=ALU.add)
                mt = comb.tile([P, 1], F32, name="mt", tag="c1")
                nc.vector.tensor_reduce(
                    out=mt[:],
                    in_=mq[:, it * QG:(it + 1) * QG].rearrange(
                        "p (a b) -> p a b", a=1),
                    axis=AXX, op=ALU.max)
                ge = comb.tile([P, QG], F32, name="ge", tag="c2")
                nc.vector.tensor_tensor(
                    out=ge[:], in0=mq[:, it * QG:(it + 1) * QG],
                    in1=mt[:].to_broadcast([P, QG]), op=ALU.is_ge)
                pick = comb.tile([P, QG], F32, name="pick", tag="c3")
                nc.vector.scalar_tensor_tensor(
                    out=pick[:], in0=ge[:], scalar=-BIGI, in1=gidx[:],
                    op0=ALU.mult, op1=ALU.add)
                mn = comb.tile([P, 1], F32, name="mn", tag="c4")
                nc.vector.tensor_reduce(
                    out=mn[:], in_=pick[:].rearrange("p (a b) -> p a b", a=1),
                    axis=AXX, op=ALU.min)
                idxf = comb.tile([P, 1], F32, name="idxf", tag="c5")
                nc.vector.tensor_scalar(
                    out=idxf[:], in0=mn[:], scalar1=BIGI, scalar2=None,
                    op0=ALU.add)
                nc.vector.tensor_copy(out=idxall[:, it:it + 1], in_=idxf[:])

                xneg = xneg_p.tile([P, D], BF16, name="xneg")
                nc.gpsimd.indirect_dma_start(
                    out=xneg[:], out_offset=None,
                    in_=io["xfb"][:, :],
                    in_offset=IndirectOffsetOnAxis(
                        ap=idxall[:, it:it + 1], axis=0),
                    bounds_check=cfg.n - 1, oob_is_err=False,
                )
                diff = diff_p.tile([P, D], BF16, name="diff")
                nc.gpsimd.tensor_tensor(
                    out=diff[:], in0=xrb_sb[:, it * D:(it + 1) * D],
                    in1=xneg[:], op=ALU.subtract,
                )
                # |diff| = max(-diff, diff), accum_out = sum -> d_an numerator
                dabs = diff_p.tile([P, D], BF16, name="dabs", tag="dabs")
                nc.vector.scalar_tensor_tensor(
                    out=dabs[:], in0=diff[:], scalar=-1.0, in1=diff[:],
                    op0=ALU.mult, op1=ALU.max,
                    accum_out=san[:, it:it + 1],
                )

                sap8 = comb.tile([P, 1], F32, name="sap8", tag="c6")
                nc.vector.tensor_reduce(
                    out=sap8[:],
                    in_=sap[:, it * QG:(it + 1) * QG].rearrange(
                        "p (a b) -> p a b", a=1),
                    axis=AXX, op=ALU.add)
                t1 = comb.tile([P, 1], F32, name="t1", tag="c7")
                nc.vector.tensor_scalar(
                    out=t1[:], in0=san[:, it:it + 1], scalar1=1.0 / D,
                    scalar2=EPS, op0=ALU.mult, op1=ALU.add)
                rec = comb.tile([P, 1], F32, name="rec", tag="c8")
                nc.vector.reciprocal(out=rec[:], in_=t1[:])
                t2 = comb.tile([P, 1], F32, name="t2", tag="c9")
                nc.vector.tensor_tensor(
                    out=t2[:], in0=sap8[:], in1=rec[:], op=ALU.mult)
                nc.vector.tensor_scalar(
                    out=lossv[:, it:it + 1], in0=t2[:],
                    scalar1=0.5 * WEIGHT / D, scalar2=None, op0=ALU.mult)

    nc.sync.dma_start(out=io["loss_part"][:, :], in_=lossv[:])
    nc.sync.dma_start(out=io["nidx"][:, :], in_=idxall[:])

    for p in reversed(list(ctxpools.values())):
        p.release()


def build(cfg: Cfg) -> bass.Bass:
    nc = bacc.Bacc("TRN2", target_bir_lowering=False, debug=False)
    io = {
        "xm": nc.dram_tensor("xm", [cfg.ns * P, cfg.kb * JS],
                             mybir.dt.float8e4, kind="ExternalInput").ap(),
        "xs": nc.dram_tensor("xs", [P, cfg.kb * cfg.r], mybir.dt.float8e4,
                             kind="ExternalInput").ap(),
        "xrb": nc.dram_tensor("xrb", [cfg.r, cfg.d], BF16,
                              kind="ExternalInput").ap(),
        "xfb": nc.dram_tensor("xfb", [cfg.n, cfg.d], BF16,
                              kind="ExternalInput").ap(),
        "m2b": nc.dram_tensor("m2b", [P, P], BF16, kind="ExternalInput").ap(),
        "maskw": nc.dram_tensor("maskw", [P, 896], F32,
                                kind="ExternalInput").ap(),
        "qoffw": nc.dram_tensor("qoffw", [P, cfg.qg], F32,
                                kind="ExternalInput").ap(),
        "loss_part": nc.dram_tensor("loss_part", [P, cfg.it], F32,
                                    kind="ExternalOutput").ap(),
        "nidx": nc.dram_tensor("nidx", [P, cfg.it], U32,
                               kind="ExternalOutput").ap(),
    }
    with tile.TileContext(nc) as tc:
        _body(tc, cfg, io)
    nc.compile()
    return nc


def make_in_maps(cfg: Cfg, x: np.ndarray) -> list[dict]:
    x = np.ascontiguousarray(x, dtype=np.float32)
    xt_q = np.ascontiguousarray(x.T.astype(ml_dtypes.float8_e4m3))
    x_bf = x.astype(ml_dtypes.bfloat16)

    m2 = np.eye(P, dtype=np.float32)
    for c in range(P // CHUNK):
        m2[c * CHUNK:(c + 1) * CHUNK, c * CHUNK:(c + 1) * CHUNK] -= 1.0 / CHUNK
    m2b = m2.astype(ml_dtypes.bfloat16)

    # [zeros(384) | group-exclusion mask(128) | zeros(384)]; slicing a
    # 512-wide window places the mask at the i-tile's own column offset.
    maskw = np.zeros((P, 896), dtype=np.float32)
    pvec = np.arange(P)
    for jcol in range(GROUP):
        maskw[pvec, 384 + (pvec // GROUP) * GROUP + jcol] = NEG_BIG

    qoffw = np.broadcast_to(
        (np.arange(cfg.qg, dtype=np.float32) * cfg.qw), (P, cfg.qg)).copy()

    in_maps = []
    for c in range(cfg.cores):
        xm_c = np.roll(xt_q, -c * cfg.r, axis=1)
        # swizzle [d, n] -> [strip, partition, kblock, js] so each SBUF
        # partition's strip slice is contiguous in DRAM
        xm_swz = np.ascontiguousarray(
            xm_c.reshape(cfg.kb, P, cfg.ns, JS).transpose(2, 1, 0, 3)
        ).reshape(cfg.ns * P, cfg.kb * JS)
        # xs: [d, r] -> [partition, kblock, r]
        xs_swz = np.ascontiguousarray(
            xm_c[:, :cfg.r].reshape(cfg.kb, P, cfg.r).transpose(1, 0, 2)
        ).reshape(P, cfg.kb * cfg.r)
        in_maps.append({
            "xm": xm_swz,
            "xs": xs_swz,
            "xrb": np.ascontiguousarray(x_bf[c * cfg.r:(c + 1) * cfg.r]),
            "xfb": np.ascontiguousarray(np.roll(x_bf, -c * cfg.r, axis=0)),
            "m2b": m2b,
            "maskw": maskw,
            "qoffw": qoffw,
        })
    return in_maps


def reduce_outputs(cfg: Cfg, results: list[dict]) -> np.ndarray:
    total = 0.0
    for res in results:
        total += float(res["loss_part"].astype(np.float64).sum())
    return np.float32(total)


def run(cfg: Cfg, x: np.ndarray, trace: bool = False):
    nc = build(cfg)
    in_maps = make_in_maps(cfg, x)
    out = run_bass_kernel_spmd(nc, in_maps, list(range(cfg.cores)), trace=trace)
    return out


def kernel(x: np.ndarray) -> np.ndarray:
    cfg = Cfg(n=8192, d=2048, cores=8)
    last_err = None
    for _ in range(3):
        try:
            out = run(cfg, x)
            return reduce_outputs(cfg, out.results)
        except Exception as e:  # transient device errors: rebuild + retry
            last_err = e
    raise last_err
